# revision 16
# baseline (speedup 1.0000x reference)
"""Distributed multi-head attention forward for 8 TRN2 NeuronCores.

Problem: B=2, N=2048, D=768, 12 heads x 64 head-dim, f32.
  qkv = x @ w_qkv + b_qkv ; per-head softmax(q k^T / 8) v ; out proj.

Sharding: core = 4*b + g (b = batch element, g = query-chunk of 512 rows).
No collectives: every core receives the FULL x^T of its batch (bf16,
host-transposed, token-rotated so its own 512 query rows sit first) and
replicates the K^T / V projections for all 2048 keys locally — on this part
the 55us+ fixed cost of a 4-core ring AllGather loses to ~60us of extra
bf16 matmuls that pipeline perfectly.

Schedule (single PE stream, everything else slotted around it):
  Q proj -> K proj ct 0 -> attention j=0..4 each interleaving the next K
  column block as PE filler (j=0 also interleaves all 16 V-projection
  steps chunk-by-chunk) -> attention j=5 -> output projection.  S runs two
  chunks ahead (PSUM: S tiles 3-deep = 6 banks + one PV accumulator pair =
  2 banks); each head pair's finalize (den -> ones-broadcast matmul ->
  reciprocal_approx_fast -> multiply, all off the Scalar engine so it does
  exps only) is deferred into chunk 0 of the next pair's loop.

Layouts: all activations transposed ([cols, tokens]) except V (natural),
everything bf16 on the wire and in SBUF; psum accumulation f32.  V carries
a per-head ones column so P@V also yields the softmax denominator; the V
bias is folded into the output bias on the host (sum(P)=1).
"""

import numpy as np

import concourse.bass as bass
import concourse.tile as tile
from concourse import bacc, mybir
from concourse.bass import ts, ds
from concourse.bass_utils import run_bass_kernel_spmd

FP = mybir.dt.float32
FR = mybir.dt.float32r
BF = mybir.dt.bfloat16

P = 128
T = 512            # query rows per core
D = 768            # model dim
H = 12             # heads
DH = 64            # head dim
VA = H * DH        # 768 v columns (softmax den comes from a ones matmul)
KEYS = 2048
DC = D // P        # 6 chunks of the contraction dim
NKC = KEYS // P    # 16 key chunks of 128
NKT = KEYS // T    # 4 key chunks of 512
SCALE = DH ** -0.5


def build_nc():
    nc = bacc.Bacc(
        "TRN2",
        target_bir_lowering=False,
        debug=False,
        enable_asserts=False,
        num_devices=8,
    )
    import os
    dbg = {}
    for name, shape in (
        ("dQT", [P, DC, T]), ("dKT", [P, DC, KEYS]),
        ("dV", [P, NKC, VA]), ("dOT", [P, DC, T]),
    ):
        if name[1:] in os.environ.get("KDBG", "").split(","):
            dbg[name[1:]] = nc.dram_tensor(name, shape, BF, kind="ExternalOutput").ap()

    xT = nc.dram_tensor("xT", [D, KEYS], BF, kind="ExternalInput").ap()
    wq = nc.dram_tensor("wq", [DC, P, DC, P], BF, kind="ExternalInput").ap()
    wk = nc.dram_tensor("wk", [DC, P, DC, P], BF, kind="ExternalInput").ap()
    wv = nc.dram_tensor("wv", [D, D], BF, kind="ExternalInput").ap()
    bq = nc.dram_tensor("bq", [P, DC], FP, kind="ExternalInput").ap()
    bk = nc.dram_tensor("bk", [P, DC], FP, kind="ExternalInput").ap()
    wo = nc.dram_tensor("wo", [D, D], BF, kind="ExternalInput").ap()
    bo = nc.dram_tensor("bo", [1, D], BF, kind="ExternalInput").ap()
    out = nc.dram_tensor("out", [T, D], FP, kind="ExternalOutput").ap()

    with tile.TileContext(nc) as tc:
        _build_body(tc, xT, wq, wk, wv, bq, bk, wo, bo, out, dbg)
    nc.compile()
    return nc


def _build_body(tc, xT_d, wq, wk, wv, bq, bk, wo, bo, out, dbg=None):
    nc = tc.nc
    Add = mybir.AluOpType.add
    Mult = mybir.AluOpType.mult
    Exp = mybir.ActivationFunctionType.Exp

    big = tc.alloc_tile_pool(name="big", bufs=1)
    stream = tc.alloc_tile_pool(name="stream", bufs=2)
    singles = tc.alloc_tile_pool(name="singles", bufs=1)
    psum = tc.alloc_tile_pool(name="psum", bufs=2, space="PSUM")

    # b2: [128, 1024] f32 = 2 psum banks; bufs=3 -> 6 banks.
    def b2(name):
        return psum.tile([P, 2 * T], FP, tag="b2", bufs=3, name=name)

    # pv: attention accumulator, 2 banks, single-buffered.
    def bpv(name):
        return psum.tile([P, 2 * T], FP, tag="pv", bufs=1, name=name)

    # ---- persistent SBUF tensors ----
    xT = big.tile([P, DC, KEYS], BF)     # x^T, all tokens (rotated)
    QT = big.tile([P, DC, T], BF)        # Q^T for own 512 rows (biased)
    KT = big.tile([P, DC, KEYS], BF)     # K^T all keys (biased)
    V = big.tile([P, NKC, VA], BF)       # V all keys (+ones cols)
    OT = big.tile([P, DC, T], BF)        # attention output, transposed
    wv_sb = big.tile([P, DC, D], BF)
    wo_sb = big.tile([P, DC, D], BF)

    # ---- constants ----
    ones_bf = singles.tile([1, DH], BF)
    nc.vector.memset(ones_bf, 1.0)
    ones_row = singles.tile([1, P], BF)   # K=1 stationary for the bias matmul
    nc.vector.memset(ones_row, 1.0)
    ones_col = singles.tile([P, 1], BF)   # key-dim reduction for softmax den
    nc.vector.memset(ones_col, 1.0)
    junk = singles.tile([P, P], BF)       # PE warm-up operand, contents unused
    nc.vector.memset(junk, 0.0)
    bq_sb = singles.tile([P, DC], FP)
    bk_sb = singles.tile([P, DC], FP)
    bo_row = singles.tile([1, D], BF)

    # ---- PE warm-up: junk matmuls with no DMA deps so the HAM un-throttles
    # and the array is at 2.4 GHz when the first real matmul's inputs land.
    warm_ps = b2("warm")
    for _ in range(44):
        nc.tensor.matmul(warm_ps[:, :P], junk, junk, start=True, stop=True)

    # ---- input DMAs: split fine-grained, spread across engine queues, in
    # consumption order (descriptor issue is ~0.6-0.8us per dma_start and
    # serializes per engine; the old single-queue scheme pushed the first
    # matmul's deps out to ~17us).
    wq_sb = big.tile([P, DC, DC, P], BF)   # [p, ct, o, c]
    wk_sb = big.tile([P, DC, DC, P], BF)
    xTr = xT_d.rearrange("(dc p) n -> p dc n", p=P)
    # wave 1: Q-proj deps (wq per-ct, x own rows per-dc, biases)
    nc.scalar.dma_start(wq_sb[:, 0], wq[0])
    nc.gpsimd.dma_start(xT[:, 0, 0:T], xTr[:, 0, 0:T])
    nc.sync.dma_start(xT[:, 1, 0:T], xTr[:, 1, 0:T])
    nc.scalar.dma_start(wq_sb[:, 1], wq[1])
    nc.gpsimd.dma_start(wq_sb[:, 2], wq[2])
    nc.sync.dma_start(xT[:, 2, 0:T], xTr[:, 2, 0:T])
    nc.scalar.dma_start(bq_sb, bq)
    nc.gpsimd.dma_start(xT[:, 3, 0:T], xTr[:, 3, 0:T])
    nc.sync.dma_start(wq_sb[:, 3], wq[3])
    nc.scalar.dma_start(bk_sb, bk)
    nc.gpsimd.dma_start(xT[:, 4, 0:T], xTr[:, 4, 0:T])
    nc.sync.dma_start(xT[:, 5, 0:T], xTr[:, 5, 0:T])
    nc.gpsimd.dma_start(wq_sb[:, 4], wq[4])
    nc.sync.dma_start(wq_sb[:, 5], wq[5])
    # wave 2: K ct0 weights + x remaining keys (kc>=1), then V/out weights
    nc.sync.dma_start(wk_sb[:, 0], wk[0])
    for dc in range(DC):
        eng = nc.gpsimd if dc % 2 else nc.sync
        eng.dma_start(xT[:, dc, T:KEYS], xT_d[ts(dc, P), T:KEYS])
    for dc in range(DC):
        eng = nc.gpsimd if dc % 2 else nc.sync
        eng.dma_start(wv_sb[:, dc, :], wv[ts(dc, P), :])
    for ct in range(1, DC):
        eng = nc.gpsimd if ct % 2 else nc.sync
        eng.dma_start(wk_sb[:, ct], wk[ct])
    for dc in range(DC):
        eng = nc.gpsimd if dc % 2 else nc.sync
        eng.dma_start(wo_sb[:, dc, :], wo[ts(dc, P), :])
    nc.gpsimd.dma_start(bo_row, bo)

    # ---- phase 1: Q^T projection; ct 0-1 upfront, the rest interleaved ----
    def q_group(ct):
        pq = b2("pq")
        for dc in range(DC):
            nc.tensor.matmul(
                pq[:, :T], wq_sb[:, ct, dc, :], xT[:, dc, 0:T],
                start=(dc == 0), stop=(dc == DC - 1),
            )
        nc.scalar.add(QT[:, ct, :], pq[:, :T], bq_sb[:, ct : ct + 1])

    for ct in range(DC):
        q_group(ct)

    # ---- phase 2: K^T projection; ct 0-1 upfront, ct 2-5 interleaved into
    # the attention loop as PE filler work.
    def k_group(ct, kc):
        pk = b2("pk")
        for dc in range(DC):
            nc.tensor.matmul(
                pk[:, :T], wk_sb[:, ct, dc, :], xT[:, dc, ts(kc, T)],
                start=(dc == 0), stop=(dc == DC - 1),
            )
        nc.vector.tensor_scalar(
            out=KT[:, ct, ts(kc, T)], in0=pk[:, :T],
            scalar1=bk_sb[:, ct : ct + 1], scalar2=None, op0=Add,
        )

    for kc in range(NKT):
        k_group(0, kc)

    # ---- phase 3+4: V projection (all keys) interleaved with attention j=0
    # V tile tt covers key chunk c=tt (128 tokens); attention consumes chunks
    # in the same order, so j=0 can run inside the V loop.
    def v_step(tt):
        pv = b2("pvproj")
        for dc in range(DC):
            for lo, sz in ((0, T), (T, D - T)):
                nc.tensor.matmul(
                    pv[:, ds(lo, sz)],
                    xT[:, dc, ts(tt, P)],
                    wv_sb[:, dc, ds(lo, sz)],
                    start=(dc == 0), stop=(dc == DC - 1),
                )
        nc.vector.tensor_copy(out=V[:, tt, :], in_=pv[:, 0:D])

    def attn_j(j, interleave_v=False, fill_k=(), fin_prev=None):
        """Attention for head pair (2j, 2j+1) over all 16 key chunks.
        Returns a finalize closure (run it one j later to pipeline).
        If interleave_v, the V-projection steps are interleaved; fill_k
        closures are spread across the chunk loop as PE filler work."""
        fill_k = list(fill_k)
        pv_acc = None  # allocated lazily at the first PV accumulation
        ps_tiles = {}

        def s_step(c):
            ps = b2(f"ps{j}_{c}")
            ps_tiles[c] = ps
            for hl, off in ((0, 0), (1, DH)):
                nc.tensor.matmul(
                    ps[:, ds(hl * T, T)],
                    KT[ds(off, DH), j, ts(c, P)],
                    QT[ds(off, DH), j, :],
                    start=True, stop=True,
                )

        # j0 (interleave_v) uses a 1-chunk S lookahead: with the V-projection
        # also allocating from b2, a 2-ahead emission makes v(c+2) wait on a
        # future exp via the 3-buffer rotation.  Pure-attention pairs use 2.
        ahead = 1 if interleave_v else 2
        for c0 in range(ahead):
            if interleave_v:
                v_step(c0)
            s_step(c0)
        for c in range(NKC):
            es = stream.tile([P, 2 * T], BF, tag="expS", bufs=5, name="es")
            nc.scalar.activation(es, ps_tiles[c][:, :], Exp, scale=SCALE)
            if c == 0 and fin_prev is not None:
                fin_prev()
            if c + ahead < NKC:
                s_step(c + ahead)
                if interleave_v:
                    v_step(c + ahead)
            if fill_k and c % 4 == 1:
                fill_k.pop(0)()
            # softmax denominator: accumulate es across chunks elementwise on
            # the (otherwise idle) GpSimd engine; one f32 ones-matmul per pair
            # then reduces the key partitions exactly.
            if c == 0:
                dacc = stream.tile([P, 2 * T], BF, tag="dacc", bufs=2, name="dacc")
                nc.gpsimd.tensor_copy(out=dacc, in_=es)
            else:
                nc.gpsimd.tensor_tensor(out=dacc, in0=dacc, in1=es, op=Add)
            if pv_acc is None:
                pv_acc = bpv(f"pv{j}")  # h0: [0:64, 0:512] b0, h1: [64:128, 512:] b1
            # col-tiled pair: both heads' PV stream concurrently through
            # separate column halves of the array (distinct banks: a start=True
            # clears has_written bank-wide, so the heads must not share one).
            nc.tensor.matmul(
                pv_acc[0:DH, 0:T],
                V[:, c, ds(2 * j * DH, DH)],
                es[:, 0:T],
                start=(c == 0), stop=(c == NKC - 1), tile_position=(0, 0),
            )
            nc.tensor.matmul(
                pv_acc[DH:P, ds(T, T)],
                V[:, c, ds((2 * j + 1) * DH, DH)],
                es[:, ds(T, T)],
                start=(c == 0), stop=(c == NKC - 1), tile_position=(0, 64),
            )

        def finalize():
            dps = b2(f"dps{j}")
            for hl in (0, 1):
                nc.tensor.matmul(
                    dps[0:1, ds(hl * T, T)], ones_col, dacc[:, ds(hl * T, T)],
                    start=True, stop=True,
                )
            den_bf = stream.tile([1, 2 * T], BF, tag="den", bufs=2, name="den_bf")
            nc.vector.tensor_copy(out=den_bf, in_=dps[0:1, :])
            bc = b2(f"bc{j}")
            for hl in (0, 1):
                nc.tensor.matmul(
                    bc[:DH, ds(hl * T, T)], ones_bf, den_bf[:, ds(hl * T, T)],
                    start=True, stop=True,
                )
            recip = stream.tile([DH, 2 * T], FP, tag="recip", bufs=2, name="recip")
            nc.vector.reciprocal_approx_fast(out=recip, in_=bc[:DH, :])
            for hl in (0, 1):
                nc.vector.tensor_tensor(
                    out=OT[ds(hl * DH, DH), j, :],
                    in0=pv_acc[ds(hl * DH, DH), ds(hl * T, T)],
                    in1=recip[:, ds(hl * T, T)], op=Mult,
                )

        return finalize

    fin = None
    for j in range(DC):
        fill_k = ()
        if j <= 4:
            ct = j + 1
            fill_k = tuple(
                (lambda ct=ct, kc=kc: k_group(ct, kc)) for kc in range(NKT)
            )
        fin = attn_j(j, interleave_v=(j == 0), fill_k=fill_k, fin_prev=fin)

    # ---- phase 6: output projection.  The dc 0-4 partial chains for the
    # first two token tiles don't need OT[:, 5], so they run before (and
    # overlap) the last head pair's finalize chain.
    def po_head(tt):
        po = b2(f"po{tt}")
        for lo, sz in ((0, T), (T, D - T)):
            # bias first (K=1 ones matmul) so the tail is just dc=5 + copy
            nc.tensor.matmul(
                po[:, ds(lo, sz)], ones_row, bo_row[:, ds(lo, sz)],
                start=True, stop=False,
            )
        for dc in range(DC - 1):
            for lo, sz in ((0, T), (T, D - T)):
                nc.tensor.matmul(
                    po[:, ds(lo, sz)],
                    OT[:, dc, ts(tt, P)],
                    wo_sb[:, dc, ds(lo, sz)],
                    start=False, stop=False,
                )
        return po

    po_tiles = {tt: po_head(tt) for tt in range(2)}
    fin()
    for tt in range(T // P):
        if tt not in po_tiles:
            po_tiles[tt] = po_head(tt)
        po = po_tiles[tt]
        for lo, sz in ((0, T), (T, D - T)):
            nc.tensor.matmul(
                po[:, ds(lo, sz)],
                OT[:, DC - 1, ts(tt, P)],
                wo_sb[:, DC - 1, ds(lo, sz)],
                start=False, stop=True,
            )
        for lo, sz in ((0, T), (T, D - T)):
            o_stage = stream.tile([P, T], FP, tag="ost", bufs=4, name="o_stage")
            cp_eng = nc.scalar if tt % 2 else nc.vector
            if cp_eng is nc.scalar:
                cp_eng.copy(out=o_stage[:, :sz], in_=po[:, ds(lo, sz)])
            else:
                cp_eng.tensor_copy(out=o_stage[:, :sz], in_=po[:, ds(lo, sz)])
            dma_eng = nc.gpsimd if (2 * tt + (lo != 0)) % 2 else nc.sync
            dma_eng.dma_start(out[ts(tt, P), ds(lo, sz)], o_stage[:, :sz])

    if dbg:
        tiles = {"QT": QT, "KT": KT, "V": V, "OT": OT}
        for name, dap in dbg.items():
            nc.sync.dma_start(dap, tiles[name])

    for pool in (psum, singles, stream, big):
        pool.release()


_CACHE = {}


def _get_nc():
    if "nc" not in _CACHE:
        _CACHE["nc"] = build_nc()
    return _CACHE["nc"]


def _prep_inputs(x, w_qkv, b_qkv, w_out, b_out):
    import ml_dtypes

    bf16 = ml_dtypes.bfloat16
    x = np.asarray(x, np.float32)
    w_qkv = np.asarray(w_qkv, np.float32)
    b_qkv = np.asarray(b_qkv, np.float32)
    w_out = np.asarray(w_out, np.float32)
    b_out = np.asarray(b_out, np.float32)

    wq_n = w_qkv[:, 0:768]
    wk_n = w_qkv[:, 768:1536]
    wv_raw = w_qkv[:, 1536:2304]
    # [p, o] layout: contiguous per-partition rows on the wire
    bq = np.ascontiguousarray(b_qkv[0:768].reshape(DC, P).T)
    bk = np.ascontiguousarray(b_qkv[768:1536].reshape(DC, P).T)
    bv_raw = b_qkv[1536:2304]

    # [ct, p, o, c] layout so the per-ct stationary DMA is contiguous
    def w_re(w):
        return np.ascontiguousarray(
            w.reshape(DC, P, DC, P).transpose(2, 1, 0, 3).astype(bf16)
        )

    wq_r = w_re(wq_n)
    wk_r = w_re(wk_n)

    wv = np.ascontiguousarray(wv_raw.astype(bf16))
    # V bias folds into the output bias: softmax rows sum to 1.
    bo_eff = np.ascontiguousarray(
        (b_out + bv_raw @ w_out).astype(bf16).reshape(1, D)
    )
    wo = np.ascontiguousarray(w_out.astype(bf16))

    in_maps = []
    for b in range(2):
        xb = x[b]
        for g in range(4):
            xrot = np.roll(xb, -g * T, axis=0)
            xTb = np.ascontiguousarray(xrot.T.astype(bf16))
            in_maps.append(
                dict(
                    xT=xTb, wq=wq_r, wk=wk_r, wv=wv, bq=bq, bk=bk,
                    wo=wo, bo=bo_eff,
                )
            )
    return in_maps


def run_on_hw(x, w_qkv, b_qkv, w_out, b_out, **kwargs):
    in_maps = _prep_inputs(x, w_qkv, b_qkv, w_out, b_out)
    res = run_bass_kernel_spmd(_get_nc(), in_maps, core_ids=list(range(8)), **kwargs)
    full = np.empty((2, 2048, D), np.float32)
    for b in range(2):
        for g in range(4):
            full[b, g * T : (g + 1) * T] = np.asarray(
                res.results[b * 4 + g]["out"], np.float32
            )
    return full, res


def kernel(x, w_qkv, b_qkv, w_out, b_out):
    full, _ = run_on_hw(x, w_qkv, b_qkv, w_out, b_out)
    return full



# revision 17
# speedup vs baseline: 1.3455x; 1.3455x over previous
"""Distributed multi-head attention forward for 8 TRN2 NeuronCores.

Problem: B=2, N=2048, D=768, 12 heads x 64 head-dim, f32.
  qkv = x @ w_qkv + b_qkv ; per-head softmax(q k^T / 8) v ; out proj.

Sharding: core = 4*b + g (b = batch element, g = query-chunk of 512 rows).
No collectives: every core receives the FULL x^T of its batch (bf16,
host-transposed, token-rotated so its own 512 query rows sit first) and
replicates the K^T / V projections for all 2048 keys locally — on this part
the 55us+ fixed cost of a 4-core ring AllGather loses to ~60us of extra
bf16 matmuls that pipeline perfectly.

Schedule (single PE stream, everything else slotted around it):
  Q proj -> K proj ct 0 -> attention j=0..4 each interleaving the next K
  column block as PE filler (j=0 also interleaves all 16 V-projection
  steps chunk-by-chunk) -> attention j=5 -> output projection.  S runs two
  chunks ahead (PSUM: S tiles 3-deep = 6 banks + one PV accumulator pair =
  2 banks); each head pair's finalize (den -> ones-broadcast matmul ->
  reciprocal_approx_fast -> multiply, all off the Scalar engine so it does
  exps only) is deferred into chunk 0 of the next pair's loop.

Layouts: all activations transposed ([cols, tokens]) except V (natural),
everything bf16 on the wire and in SBUF; psum accumulation f32.  V carries
a per-head ones column so P@V also yields the softmax denominator; the V
bias is folded into the output bias on the host (sum(P)=1).
"""

import numpy as np

import concourse.bass as bass
import concourse.tile as tile
from concourse import bacc, mybir
from concourse.bass import ts, ds
from concourse.bass_utils import run_bass_kernel_spmd

FP = mybir.dt.float32
FR = mybir.dt.float32r
BF = mybir.dt.bfloat16

P = 128
T = 512            # query rows per core
D = 768            # model dim
H = 12             # heads
DH = 64            # head dim
VA = H * DH        # 768 v columns (softmax den comes from a ones matmul)
KEYS = 2048
DC = D // P        # 6 chunks of the contraction dim
NKC = KEYS // P    # 16 key chunks of 128
NKT = KEYS // T    # 4 key chunks of 512
SCALE = DH ** -0.5


def build_nc():
    nc = bacc.Bacc(
        "TRN2",
        target_bir_lowering=False,
        debug=False,
        enable_asserts=False,
        num_devices=8,
    )
    import os
    dbg = {}
    for name, shape in (
        ("dQT", [P, DC, T]), ("dKT", [P, DC, KEYS]),
        ("dV", [P, NKC, VA]), ("dOT", [P, DC, T]),
    ):
        if name[1:] in os.environ.get("KDBG", "").split(","):
            dbg[name[1:]] = nc.dram_tensor(name, shape, BF, kind="ExternalOutput").ap()

    xT = nc.dram_tensor("xT", [D, KEYS], BF, kind="ExternalInput").ap()
    wq = nc.dram_tensor("wq", [DC, P, DC, P], BF, kind="ExternalInput").ap()
    wk = nc.dram_tensor("wk", [DC, P, DC, P], BF, kind="ExternalInput").ap()
    wv = nc.dram_tensor("wv", [D, D], BF, kind="ExternalInput").ap()
    bq = nc.dram_tensor("bq", [P, DC], FP, kind="ExternalInput").ap()
    bk = nc.dram_tensor("bk", [P, DC], FP, kind="ExternalInput").ap()
    wo = nc.dram_tensor("wo", [D, D], BF, kind="ExternalInput").ap()
    bo = nc.dram_tensor("bo", [1, D], BF, kind="ExternalInput").ap()
    out = nc.dram_tensor("out", [T, D], FP, kind="ExternalOutput").ap()

    with tile.TileContext(nc) as tc:
        _build_body(tc, xT, wq, wk, wv, bq, bk, wo, bo, out, dbg)
    nc.compile()
    return nc


def _build_body(tc, xT_d, wq, wk, wv, bq, bk, wo, bo, out, dbg=None):
    nc = tc.nc
    Add = mybir.AluOpType.add
    Mult = mybir.AluOpType.mult
    Exp = mybir.ActivationFunctionType.Exp

    big = tc.alloc_tile_pool(name="big", bufs=1)
    stream = tc.alloc_tile_pool(name="stream", bufs=2)
    singles = tc.alloc_tile_pool(name="singles", bufs=1)
    psum = tc.alloc_tile_pool(name="psum", bufs=2, space="PSUM")

    # b2: [128, 1024] f32 = 2 psum banks; bufs=3 -> 6 banks.
    def b2(name):
        return psum.tile([P, 2 * T], FP, tag="b2", bufs=3, name=name)

    # pv: attention accumulator, 2 banks, single-buffered.
    def bpv(name):
        return psum.tile([P, 2 * T], FP, tag="pv", bufs=1, name=name)

    # ---- persistent SBUF tensors ----
    xT = big.tile([P, DC, KEYS], BF)     # x^T, all tokens (rotated)
    QT = big.tile([P, DC, T], BF)        # Q^T for own 512 rows (biased)
    KT = big.tile([P, DC, KEYS], BF)     # K^T all keys (biased)
    V = big.tile([P, NKC, VA], BF)       # V all keys (+ones cols)
    OT = big.tile([P, DC, T], BF)        # attention output, transposed
    wv_sb = big.tile([P, DC, D], BF)
    wo_sb = big.tile([P, DC, D], BF)

    # ---- constants ----
    ones_bf = singles.tile([1, DH], BF)
    nc.vector.memset(ones_bf, 1.0)
    ones_row = singles.tile([1, P], BF)   # K=1 stationary for the bias matmul
    nc.vector.memset(ones_row, 1.0)
    ones_col = singles.tile([P, 1], BF)   # key-dim reduction for softmax den
    nc.vector.memset(ones_col, 1.0)
    junk = singles.tile([P, P], BF)       # PE warm-up operand, contents unused
    nc.vector.memset(junk, 0.0)
    bq_sb = singles.tile([P, DC], FP)
    bk_sb = singles.tile([P, DC], FP)
    bo_row = singles.tile([1, D], BF)

    # ---- PE warm-up: junk matmuls with no DMA deps so the HAM un-throttles
    # and the array is at 2.4 GHz when the first real matmul's inputs land.
    warm_ps = b2("warm")
    for _ in range(44):
        nc.tensor.matmul(warm_ps[:, :P], junk, junk, start=True, stop=True)

    # ---- input DMAs: split fine-grained, spread across engine queues, in
    # consumption order (descriptor issue is ~0.6-0.8us per dma_start and
    # serializes per engine; the old single-queue scheme pushed the first
    # matmul's deps out to ~17us).
    wq_sb = big.tile([P, DC, DC, P], BF)   # [p, ct, o, c]
    wk_sb = big.tile([P, DC, DC, P], BF)
    xTr = xT_d.rearrange("(dc p) n -> p dc n", p=P)
    # wave 1: Q-proj deps (wq per-ct, x own rows per-dc, biases)
    nc.scalar.dma_start(wq_sb[:, 0], wq[0])
    nc.gpsimd.dma_start(xT[:, 0, 0:T], xTr[:, 0, 0:T])
    nc.sync.dma_start(xT[:, 1, 0:T], xTr[:, 1, 0:T])
    nc.scalar.dma_start(wq_sb[:, 1], wq[1])
    nc.gpsimd.dma_start(wq_sb[:, 2], wq[2])
    nc.sync.dma_start(xT[:, 2, 0:T], xTr[:, 2, 0:T])
    nc.scalar.dma_start(bq_sb, bq)
    nc.gpsimd.dma_start(xT[:, 3, 0:T], xTr[:, 3, 0:T])
    nc.sync.dma_start(wq_sb[:, 3], wq[3])
    nc.scalar.dma_start(bk_sb, bk)
    nc.gpsimd.dma_start(xT[:, 4, 0:T], xTr[:, 4, 0:T])
    nc.sync.dma_start(xT[:, 5, 0:T], xTr[:, 5, 0:T])
    nc.gpsimd.dma_start(wq_sb[:, 4], wq[4])
    nc.sync.dma_start(wq_sb[:, 5], wq[5])
    # wave 2: K ct0 weights + x remaining keys (kc>=1), then V/out weights
    nc.sync.dma_start(wk_sb[:, 0], wk[0])
    for dc in range(DC):
        eng = nc.gpsimd if dc % 2 else nc.sync
        eng.dma_start(xT[:, dc, T:KEYS], xT_d[ts(dc, P), T:KEYS])
    for dc in range(DC):
        eng = nc.gpsimd if dc % 2 else nc.sync
        eng.dma_start(wv_sb[:, dc, :], wv[ts(dc, P), :])
    for ct in range(1, DC):
        eng = nc.gpsimd if ct % 2 else nc.sync
        eng.dma_start(wk_sb[:, ct], wk[ct])
    for dc in range(DC):
        eng = nc.gpsimd if dc % 2 else nc.sync
        eng.dma_start(wo_sb[:, dc, :], wo[ts(dc, P), :])
    nc.gpsimd.dma_start(bo_row, bo)

    # ---- phase 1: Q^T projection; ct 0-1 upfront, the rest interleaved ----
    def q_group(ct):
        pq = b2("pq")
        for dc in range(DC):
            nc.tensor.matmul(
                pq[:, :T], wq_sb[:, ct, dc, :], xT[:, dc, 0:T],
                start=(dc == 0), stop=(dc == DC - 1),
            )
        nc.scalar.add(QT[:, ct, :], pq[:, :T], bq_sb[:, ct : ct + 1])

    for ct in range(DC):
        q_group(ct)

    # ---- phase 2: K^T projection; ct 0-1 upfront, ct 2-5 interleaved into
    # the attention loop as PE filler work.
    def k_group(ct, kc):
        pk = b2("pk")
        for dc in range(DC):
            nc.tensor.matmul(
                pk[:, :T], wk_sb[:, ct, dc, :], xT[:, dc, ts(kc, T)],
                start=(dc == 0), stop=(dc == DC - 1),
            )
        nc.vector.tensor_scalar(
            out=KT[:, ct, ts(kc, T)], in0=pk[:, :T],
            scalar1=bk_sb[:, ct : ct + 1], scalar2=None, op0=Add,
        )

    for kc in range(NKT):
        k_group(0, kc)

    # ---- phase 3+4: V projection (all keys) interleaved with attention j=0
    # V tile tt covers key chunk c=tt (128 tokens); attention consumes chunks
    # in the same order, so j=0 can run inside the V loop.
    def v_step(tt):
        pv = b2("pvproj")
        for dc in range(DC):
            for lo, sz in ((0, T), (T, D - T)):
                nc.tensor.matmul(
                    pv[:, ds(lo, sz)],
                    xT[:, dc, ts(tt, P)],
                    wv_sb[:, dc, ds(lo, sz)],
                    start=(dc == 0), stop=(dc == DC - 1),
                )
        nc.vector.tensor_copy(out=V[:, tt, :], in_=pv[:, 0:D])

    def attn_j(j, interleave_v=False, fill_k=(), fin_prev=None):
        """Attention for head pair (2j, 2j+1) over all 16 key chunks.
        Returns a finalize closure (run it one j later to pipeline).
        If interleave_v, the V-projection steps are interleaved; fill_k
        closures are spread across the chunk loop as PE filler work."""
        fill_k = list(fill_k)
        pv_acc = None  # allocated lazily at the first PV accumulation
        ps_tiles = {}

        def s_step(c):
            ps = b2(f"ps{j}_{c}")
            ps_tiles[c] = ps
            for hl, off in ((0, 0), (1, DH)):
                nc.tensor.matmul(
                    ps[:, ds(hl * T, T)],
                    KT[ds(off, DH), j, ts(c, P)],
                    QT[ds(off, DH), j, :],
                    start=True, stop=True,
                )

        # j0 (interleave_v) uses a 1-chunk S lookahead: with the V-projection
        # also allocating from b2, a 2-ahead emission makes v(c+2) wait on a
        # future exp via the 3-buffer rotation.  Pure-attention pairs use 2.
        ahead = 1 if interleave_v else 2
        for c0 in range(ahead):
            if interleave_v:
                v_step(c0)
            s_step(c0)
        for c in range(NKC):
            es = stream.tile([P, 2 * T], BF, tag="expS", bufs=5, name="es")
            nc.scalar.activation(es, ps_tiles[c][:, :], Exp, scale=SCALE)
            if c == 0 and fin_prev is not None:
                fin_prev()
            if c + ahead < NKC:
                s_step(c + ahead)
                if interleave_v:
                    v_step(c + ahead)
            if fill_k and c % 4 == 1:
                fill_k.pop(0)()
            # softmax denominator: accumulate es across chunks elementwise on
            # the (otherwise idle) GpSimd engine; one f32 ones-matmul per pair
            # then reduces the key partitions exactly.
            if c == 0:
                dacc = stream.tile([P, 2 * T], BF, tag="dacc", bufs=2, name="dacc")
                nc.vector.tensor_copy(out=dacc, in_=es)
            else:
                nc.vector.tensor_tensor(out=dacc, in0=dacc, in1=es, op=Add)
            if pv_acc is None:
                pv_acc = bpv(f"pv{j}")  # h0: [0:64, 0:512] b0, h1: [64:128, 512:] b1
            # col-tiled pair: both heads' PV stream concurrently through
            # separate column halves of the array (distinct banks: a start=True
            # clears has_written bank-wide, so the heads must not share one).
            nc.tensor.matmul(
                pv_acc[0:DH, 0:T],
                V[:, c, ds(2 * j * DH, DH)],
                es[:, 0:T],
                start=(c == 0), stop=(c == NKC - 1), tile_position=(0, 0),
            )
            nc.tensor.matmul(
                pv_acc[DH:P, ds(T, T)],
                V[:, c, ds((2 * j + 1) * DH, DH)],
                es[:, ds(T, T)],
                start=(c == 0), stop=(c == NKC - 1), tile_position=(0, 64),
            )

        def finalize():
            dps = b2(f"dps{j}")
            for hl in (0, 1):
                nc.tensor.matmul(
                    dps[0:1, ds(hl * T, T)], ones_col, dacc[:, ds(hl * T, T)],
                    start=True, stop=True,
                )
            den_bf = stream.tile([1, 2 * T], BF, tag="den", bufs=2, name="den_bf")
            nc.vector.tensor_copy(out=den_bf, in_=dps[0:1, :])
            bc = b2(f"bc{j}")
            for hl in (0, 1):
                nc.tensor.matmul(
                    bc[:DH, ds(hl * T, T)], ones_bf, den_bf[:, ds(hl * T, T)],
                    start=True, stop=True,
                )
            recip = stream.tile([DH, 2 * T], FP, tag="recip", bufs=2, name="recip")
            nc.vector.reciprocal_approx_fast(out=recip, in_=bc[:DH, :])
            for hl in (0, 1):
                nc.vector.tensor_tensor(
                    out=OT[ds(hl * DH, DH), j, :],
                    in0=pv_acc[ds(hl * DH, DH), ds(hl * T, T)],
                    in1=recip[:, ds(hl * T, T)], op=Mult,
                )

        return finalize

    fin = None
    for j in range(DC):
        fill_k = ()
        if j <= 4:
            ct = j + 1
            fill_k = tuple(
                (lambda ct=ct, kc=kc: k_group(ct, kc)) for kc in range(NKT)
            )
        fin = attn_j(j, interleave_v=(j == 0), fill_k=fill_k, fin_prev=fin)

    # ---- phase 6: output projection.  The dc 0-4 partial chains for the
    # first two token tiles don't need OT[:, 5], so they run before (and
    # overlap) the last head pair's finalize chain.
    def po_head(tt):
        po = b2(f"po{tt}")
        for lo, sz in ((0, T), (T, D - T)):
            # bias first (K=1 ones matmul) so the tail is just dc=5 + copy
            nc.tensor.matmul(
                po[:, ds(lo, sz)], ones_row, bo_row[:, ds(lo, sz)],
                start=True, stop=False,
            )
        for dc in range(DC - 1):
            for lo, sz in ((0, T), (T, D - T)):
                nc.tensor.matmul(
                    po[:, ds(lo, sz)],
                    OT[:, dc, ts(tt, P)],
                    wo_sb[:, dc, ds(lo, sz)],
                    start=False, stop=False,
                )
        return po

    po_tiles = {tt: po_head(tt) for tt in range(2)}
    fin()
    for tt in range(T // P):
        if tt not in po_tiles:
            po_tiles[tt] = po_head(tt)
        po = po_tiles[tt]
        for lo, sz in ((0, T), (T, D - T)):
            nc.tensor.matmul(
                po[:, ds(lo, sz)],
                OT[:, DC - 1, ts(tt, P)],
                wo_sb[:, DC - 1, ds(lo, sz)],
                start=False, stop=True,
            )
        for lo, sz in ((0, T), (T, D - T)):
            o_stage = stream.tile([P, T], FP, tag="ost", bufs=4, name="o_stage")
            cp_eng = nc.scalar if tt % 2 else nc.vector
            if cp_eng is nc.scalar:
                cp_eng.copy(out=o_stage[:, :sz], in_=po[:, ds(lo, sz)])
            else:
                cp_eng.tensor_copy(out=o_stage[:, :sz], in_=po[:, ds(lo, sz)])
            dma_eng = nc.gpsimd if (2 * tt + (lo != 0)) % 2 else nc.sync
            dma_eng.dma_start(out[ts(tt, P), ds(lo, sz)], o_stage[:, :sz])

    if dbg:
        tiles = {"QT": QT, "KT": KT, "V": V, "OT": OT}
        for name, dap in dbg.items():
            nc.sync.dma_start(dap, tiles[name])

    for pool in (psum, singles, stream, big):
        pool.release()


_CACHE = {}


def _get_nc():
    if "nc" not in _CACHE:
        _CACHE["nc"] = build_nc()
    return _CACHE["nc"]


def _prep_inputs(x, w_qkv, b_qkv, w_out, b_out):
    import ml_dtypes

    bf16 = ml_dtypes.bfloat16
    x = np.asarray(x, np.float32)
    w_qkv = np.asarray(w_qkv, np.float32)
    b_qkv = np.asarray(b_qkv, np.float32)
    w_out = np.asarray(w_out, np.float32)
    b_out = np.asarray(b_out, np.float32)

    wq_n = w_qkv[:, 0:768]
    wk_n = w_qkv[:, 768:1536]
    wv_raw = w_qkv[:, 1536:2304]
    # [p, o] layout: contiguous per-partition rows on the wire
    bq = np.ascontiguousarray(b_qkv[0:768].reshape(DC, P).T)
    bk = np.ascontiguousarray(b_qkv[768:1536].reshape(DC, P).T)
    bv_raw = b_qkv[1536:2304]

    # [ct, p, o, c] layout so the per-ct stationary DMA is contiguous
    def w_re(w):
        return np.ascontiguousarray(
            w.reshape(DC, P, DC, P).transpose(2, 1, 0, 3).astype(bf16)
        )

    wq_r = w_re(wq_n)
    wk_r = w_re(wk_n)

    wv = np.ascontiguousarray(wv_raw.astype(bf16))
    # V bias folds into the output bias: softmax rows sum to 1.
    bo_eff = np.ascontiguousarray(
        (b_out + bv_raw @ w_out).astype(bf16).reshape(1, D)
    )
    wo = np.ascontiguousarray(w_out.astype(bf16))

    in_maps = []
    for b in range(2):
        xb = x[b]
        for g in range(4):
            xrot = np.roll(xb, -g * T, axis=0)
            xTb = np.ascontiguousarray(xrot.T.astype(bf16))
            in_maps.append(
                dict(
                    xT=xTb, wq=wq_r, wk=wk_r, wv=wv, bq=bq, bk=bk,
                    wo=wo, bo=bo_eff,
                )
            )
    return in_maps


def run_on_hw(x, w_qkv, b_qkv, w_out, b_out, **kwargs):
    in_maps = _prep_inputs(x, w_qkv, b_qkv, w_out, b_out)
    res = run_bass_kernel_spmd(_get_nc(), in_maps, core_ids=list(range(8)), **kwargs)
    full = np.empty((2, 2048, D), np.float32)
    for b in range(2):
        for g in range(4):
            full[b, g * T : (g + 1) * T] = np.asarray(
                res.results[b * 4 + g]["out"], np.float32
            )
    return full, res


def kernel(x, w_qkv, b_qkv, w_out, b_out):
    full, _ = run_on_hw(x, w_qkv, b_qkv, w_out, b_out)
    return full



# revision 20
# speedup vs baseline: 1.3741x; 1.0212x over previous
"""Distributed multi-head attention forward for 8 TRN2 NeuronCores.

Problem: B=2, N=2048, D=768, 12 heads x 64 head-dim, f32.
  qkv = x @ w_qkv + b_qkv ; per-head softmax(q k^T / 8) v ; out proj.

Sharding: core = 4*b + g (b = batch element, g = query-chunk of 512 rows).
No collectives: every core receives the FULL x^T of its batch (bf16,
host-transposed, token-rotated so its own 512 query rows sit first) and
replicates the K^T / V projections for all 2048 keys locally — on this part
the 55us+ fixed cost of a 4-core ring AllGather loses to ~60us of extra
bf16 matmuls that pipeline perfectly.

Schedule (single PE stream, everything else slotted around it):
  Q proj -> K proj ct 0 -> attention j=0..4 each interleaving the next K
  column block as PE filler (j=0 also interleaves all 16 V-projection
  steps chunk-by-chunk) -> attention j=5 -> output projection.  S runs two
  chunks ahead (PSUM: S tiles 3-deep = 6 banks + one PV accumulator pair =
  2 banks); each head pair's finalize (den -> ones-broadcast matmul ->
  reciprocal_approx_fast -> multiply, all off the Scalar engine so it does
  exps only) is deferred into chunk 0 of the next pair's loop.

Layouts: all activations transposed ([cols, tokens]) except V (natural),
everything bf16 on the wire and in SBUF; psum accumulation f32.  V carries
a per-head ones column so P@V also yields the softmax denominator; the V
bias is folded into the output bias on the host (sum(P)=1).
"""

import numpy as np

import concourse.bass as bass
import concourse.tile as tile
from concourse import bacc, mybir
from concourse.bass import ts, ds
from concourse.bass_utils import run_bass_kernel_spmd

FP = mybir.dt.float32
FR = mybir.dt.float32r
BF = mybir.dt.bfloat16

P = 128
T = 512            # query rows per core
D = 768            # model dim
H = 12             # heads
DH = 64            # head dim
VA = H * (DH + 1)  # 780: v columns + per-head ones column (softmax den)
VH = 6 * (DH + 1)  # one half (6 heads) of the V columns
KEYS = 2048
DC = D // P        # 6 chunks of the contraction dim
NKC = KEYS // P    # 16 key chunks of 128
NKT = KEYS // T    # 4 key chunks of 512
SCALE = DH ** -0.5


def build_nc():
    nc = bacc.Bacc(
        "TRN2",
        target_bir_lowering=False,
        debug=False,
        enable_asserts=False,
        num_devices=8,
    )
    import os
    dbg = {}
    for name, shape in (
        ("dQT", [P, DC, T]), ("dKT", [P, DC, KEYS]),
        ("dV", [P, NKC, VA]), ("dOT", [P, DC, T]),
    ):
        if name[1:] in os.environ.get("KDBG", "").split(","):
            dbg[name[1:]] = nc.dram_tensor(name, shape, BF, kind="ExternalOutput").ap()

    xT = nc.dram_tensor("xT", [D, KEYS], BF, kind="ExternalInput").ap()
    wq = nc.dram_tensor("wq", [DC, P, DC, P], BF, kind="ExternalInput").ap()
    wk = nc.dram_tensor("wk", [DC, P, DC, P], BF, kind="ExternalInput").ap()
    wv = nc.dram_tensor("wv", [D, D], BF, kind="ExternalInput").ap()
    bq = nc.dram_tensor("bq", [P, DC], FP, kind="ExternalInput").ap()
    bk = nc.dram_tensor("bk", [P, DC], FP, kind="ExternalInput").ap()
    wo = nc.dram_tensor("wo", [D, D], BF, kind="ExternalInput").ap()
    bo = nc.dram_tensor("bo", [1, D], BF, kind="ExternalInput").ap()
    out = nc.dram_tensor("out", [T, D], FP, kind="ExternalOutput").ap()

    with tile.TileContext(nc) as tc:
        _build_body(tc, xT, wq, wk, wv, bq, bk, wo, bo, out, dbg)
    nc.compile()
    return nc


def _build_body(tc, xT_d, wq, wk, wv, bq, bk, wo, bo, out, dbg=None):
    nc = tc.nc
    Add = mybir.AluOpType.add
    Mult = mybir.AluOpType.mult
    Exp = mybir.ActivationFunctionType.Exp

    big = tc.alloc_tile_pool(name="big", bufs=1)
    stream = tc.alloc_tile_pool(name="stream", bufs=2)
    singles = tc.alloc_tile_pool(name="singles", bufs=1)
    psum = tc.alloc_tile_pool(name="psum", bufs=2, space="PSUM")

    # b2: [128, 1024] f32 = 2 psum banks; bufs=3 -> 6 banks.
    def b2(name):
        return psum.tile([P, 2 * T], FP, tag="b2", bufs=3, name=name)

    # pv: attention accumulator, 2 banks, single-buffered.
    def bpv(name):
        return psum.tile([P, 2 * T], FP, tag="pv", bufs=1, name=name)

    # ---- persistent SBUF tensors ----
    xT = big.tile([P, DC, KEYS], BF)     # x^T, all tokens (rotated)
    QT = big.tile([P, DC, T], BF)        # Q^T for own 512 rows (biased)
    KT = big.tile([P, DC, KEYS], BF)     # K^T all keys (biased)
    V = big.tile([P, NKC, VA], BF)       # V all keys (+ones cols)
    OT = big.tile([P, DC, T], BF)        # attention output, transposed
    wv_sb = big.tile([P, DC, D], BF)
    wo_sb = big.tile([P, DC, D], BF)

    # ---- constants ----
    ones_bf = singles.tile([1, DH], BF)
    nc.vector.memset(ones_bf, 1.0)
    ones_row = singles.tile([1, P], BF)   # K=1 stationary for the bias matmul
    nc.vector.memset(ones_row, 1.0)

    junk = singles.tile([P, P], BF)       # PE warm-up operand, contents unused
    nc.vector.memset(junk, 0.0)
    bq_sb = singles.tile([P, DC], FP)
    bk_sb = singles.tile([P, DC], FP)
    bo_row = singles.tile([1, D], BF)

    # ---- PE warm-up: junk matmuls with no DMA deps so the HAM un-throttles
    # and the array is at 2.4 GHz when the first real matmul's inputs land.
    warm_ps = b2("warm")
    for _ in range(44):
        nc.tensor.matmul(warm_ps[:, :P], junk, junk, start=True, stop=True)

    # ---- input DMAs: split fine-grained, spread across engine queues, in
    # consumption order (descriptor issue is ~0.6-0.8us per dma_start and
    # serializes per engine; the old single-queue scheme pushed the first
    # matmul's deps out to ~17us).
    wq_sb = big.tile([P, DC, DC, P], BF)   # [p, ct, o, c]
    wk_sb = big.tile([P, DC, DC, P], BF)
    xTr = xT_d.rearrange("(dc p) n -> p dc n", p=P)
    # wave 1: Q-proj deps (wq per-ct, x own rows per-dc, biases)
    nc.scalar.dma_start(wq_sb[:, 0], wq[0])
    nc.gpsimd.dma_start(xT[:, 0, 0:T], xTr[:, 0, 0:T])
    nc.sync.dma_start(xT[:, 1, 0:T], xTr[:, 1, 0:T])
    nc.scalar.dma_start(wq_sb[:, 1], wq[1])
    nc.gpsimd.dma_start(wq_sb[:, 2], wq[2])
    nc.sync.dma_start(xT[:, 2, 0:T], xTr[:, 2, 0:T])
    nc.scalar.dma_start(bq_sb, bq)
    nc.gpsimd.dma_start(xT[:, 3, 0:T], xTr[:, 3, 0:T])
    nc.sync.dma_start(wq_sb[:, 3], wq[3])
    nc.scalar.dma_start(bk_sb, bk)
    nc.gpsimd.dma_start(xT[:, 4, 0:T], xTr[:, 4, 0:T])
    nc.sync.dma_start(xT[:, 5, 0:T], xTr[:, 5, 0:T])
    nc.gpsimd.dma_start(wq_sb[:, 4], wq[4])
    nc.sync.dma_start(wq_sb[:, 5], wq[5])
    # wave 2: K ct0 weights + x remaining keys (kc>=1), then V/out weights
    nc.sync.dma_start(wk_sb[:, 0], wk[0])
    for dc in range(DC):
        eng = nc.gpsimd if dc % 2 else nc.sync
        eng.dma_start(xT[:, dc, T:KEYS], xT_d[ts(dc, P), T:KEYS])
    for dc in range(DC):
        eng = nc.gpsimd if dc % 2 else nc.sync
        eng.dma_start(wv_sb[:, dc, :], wv[ts(dc, P), :])
    for ct in range(1, DC):
        eng = nc.gpsimd if ct % 2 else nc.sync
        eng.dma_start(wk_sb[:, ct], wk[ct])
    for dc in range(DC):
        eng = nc.gpsimd if dc % 2 else nc.sync
        eng.dma_start(wo_sb[:, dc, :], wo[ts(dc, P), :])
    nc.gpsimd.dma_start(bo_row, bo)

    # ---- phase 1: Q^T projection; ct 0-1 upfront, the rest interleaved ----
    def q_group(ct):
        pq = b2("pq")
        for dc in range(DC):
            nc.tensor.matmul(
                pq[:, :T], wq_sb[:, ct, dc, :], xT[:, dc, 0:T],
                start=(dc == 0), stop=(dc == DC - 1),
            )
        nc.scalar.add(QT[:, ct, :], pq[:, :T], bq_sb[:, ct : ct + 1])

    for ct in range(DC):
        q_group(ct)

    # ---- phase 2: K^T projection; ct 0-1 upfront, ct 2-5 interleaved into
    # the attention loop as PE filler work.
    def k_group(ct, kc):
        pk = b2("pk")
        for dc in range(DC):
            nc.tensor.matmul(
                pk[:, :T], wk_sb[:, ct, dc, :], xT[:, dc, ts(kc, T)],
                start=(dc == 0), stop=(dc == DC - 1),
            )
        nc.vector.tensor_scalar(
            out=KT[:, ct, ts(kc, T)], in0=pk[:, :T],
            scalar1=bk_sb[:, ct : ct + 1], scalar2=None, op0=Add,
        )

    k_group(0, 0)

    # ---- phase 3+4: V projection, split in half by head group: pair j only
    # consumes its own heads' V columns, so heads 0-5 are produced just-in-time
    # inside pair 0's chunk loop and heads 6-11 inside pair 1's.  This halves
    # the V overhang of the first (V-bound) pair and hides the second half in
    # pair 1, whose chunks are otherwise exp-bound.
    def v_step(tt, half):
        pv = b2("pvproj")
        lo = half * (D // 2)
        for dc in range(DC):
            nc.tensor.matmul(
                pv[:, 0 : D // 2],
                xT[:, dc, ts(tt, P)],
                wv_sb[:, dc, ds(lo, D // 2)],
                start=(dc == 0), stop=(dc == DC - 1),
            )
        dst = V[:, tt, ds(half * VH, VH)].rearrange("p (h d1) -> p h d1", d1=DH + 1)
        nc.vector.tensor_copy(
            out=dst[:, :, 0:DH],
            in_=pv[:, 0 : D // 2].rearrange("p (h d) -> p h d", d=DH),
        )
        nc.vector.memset(dst[:, :, DH], 1.0)

    def attn_j(j, v_half=None, fill_k=(), fill_mod=4, fin_prev=None):
        """Attention for head pair (2j, 2j+1) over all 16 key chunks.
        Returns a finalize closure (run it one j later to pipeline).
        If v_half is set, that half of the V-projection is interleaved;
        fill_k closures are spread across the chunk loop as PE filler."""
        fill_k = list(fill_k)
        pv_acc = None  # allocated lazily at the first PV accumulation
        ps_tiles = {}

        def s_step(c):
            ps = b2(f"ps{j}_{c}")
            ps_tiles[c] = ps
            for hl, off in ((0, 0), (1, DH)):
                nc.tensor.matmul(
                    ps[:, ds(hl * T, T)],
                    KT[ds(off, DH), j, ts(c, P)],
                    QT[ds(off, DH), j, :],
                    start=True, stop=True,
                )

        # V-interleaved pairs use a 1-chunk S lookahead: with the V-projection
        # also allocating from b2, a 2-ahead emission makes v(c+2) wait on a
        # future exp via the 3-buffer rotation.  Pure-attention pairs use 2.
        ahead = 1 if v_half is not None else 2
        for c0 in range(ahead):
            if v_half is not None:
                v_step(c0, v_half)
            s_step(c0)
        for c in range(NKC):
            es = stream.tile([P, 2 * T], BF, tag="expS", bufs=5, name="es")
            nc.scalar.activation(es, ps_tiles[c][:, :], Exp, scale=SCALE)
            if c == 0 and fin_prev is not None:
                fin_prev()
            if c + ahead < NKC:
                s_step(c + ahead)
                if v_half is not None:
                    v_step(c + ahead, v_half)
            if fill_k and c % fill_mod == 1:
                fill_k.pop(0)()
            if pv_acc is None:
                pv_acc = bpv(f"pv{j}")  # h0: [:65, :512], h1: [:65, 512:]
            for hl in (0, 1):
                nc.tensor.matmul(
                    pv_acc[: DH + 1, ds(hl * T, T)],
                    V[:, c, ds((2 * j + hl) * (DH + 1), DH + 1)],
                    es[:, ds(hl * T, T)],
                    start=(c == 0), stop=(c == NKC - 1),
                )

        def finalize():
            den_bf = stream.tile([1, 2 * T], BF, tag="den", bufs=2, name="den_bf")
            nc.vector.tensor_copy(out=den_bf, in_=pv_acc[DH : DH + 1, :])
            bc = b2(f"bc{j}")
            for hl in (0, 1):
                nc.tensor.matmul(
                    bc[:DH, ds(hl * T, T)], ones_bf, den_bf[:, ds(hl * T, T)],
                    start=True, stop=True,
                )
            recip = stream.tile([DH, 2 * T], FP, tag="recip", bufs=2, name="recip")
            nc.vector.reciprocal_approx_fast(out=recip, in_=bc[:DH, :])
            for hl in (0, 1):
                nc.vector.tensor_tensor(
                    out=OT[ds(hl * DH, DH), j, :],
                    in0=pv_acc[:DH, ds(hl * T, T)],
                    in1=recip[:, ds(hl * T, T)], op=Mult,
                )

        return finalize

    fin = None
    for j in range(DC):
        if j == 0:
            # k ct0 kc1-3 first (pair 0's own S consumes them), then ct1
            fill_k = tuple(
                (lambda kc=kc: k_group(0, kc)) for kc in range(1, NKT)
            ) + tuple(
                (lambda kc=kc: k_group(1, kc)) for kc in range(NKT)
            )
            fin = attn_j(j, v_half=0, fill_k=fill_k, fill_mod=2, fin_prev=fin)
        elif j == 1:
            fill_k = tuple(
                (lambda kc=kc: k_group(2, kc)) for kc in range(NKT)
            )
            fin = attn_j(j, v_half=1, fill_k=fill_k, fill_mod=4, fin_prev=fin)
        else:
            fill_k = ()
            if j <= 4:
                ct = j + 1
                fill_k = tuple(
                    (lambda ct=ct, kc=kc: k_group(ct, kc)) for kc in range(NKT)
                )
            fin = attn_j(j, fill_k=fill_k, fin_prev=fin)

    # ---- phase 6: output projection.  The dc 0-4 partial chains for the
    # first two token tiles don't need OT[:, 5], so they run before (and
    # overlap) the last head pair's finalize chain.
    def po_head(tt):
        po = b2(f"po{tt}")
        for lo, sz in ((0, T), (T, D - T)):
            # bias first (K=1 ones matmul) so the tail is just dc=5 + copy
            nc.tensor.matmul(
                po[:, ds(lo, sz)], ones_row, bo_row[:, ds(lo, sz)],
                start=True, stop=False,
            )
        for dc in range(DC - 1):
            for lo, sz in ((0, T), (T, D - T)):
                nc.tensor.matmul(
                    po[:, ds(lo, sz)],
                    OT[:, dc, ts(tt, P)],
                    wo_sb[:, dc, ds(lo, sz)],
                    start=False, stop=False,
                )
        return po

    po_tiles = {tt: po_head(tt) for tt in range(2)}
    fin()
    for tt in range(T // P):
        if tt not in po_tiles:
            po_tiles[tt] = po_head(tt)
        po = po_tiles[tt]
        for lo, sz in ((0, T), (T, D - T)):
            nc.tensor.matmul(
                po[:, ds(lo, sz)],
                OT[:, DC - 1, ts(tt, P)],
                wo_sb[:, DC - 1, ds(lo, sz)],
                start=False, stop=True,
            )
        for lo, sz in ((0, T), (T, D - T)):
            o_stage = stream.tile([P, T], FP, tag="ost", bufs=4, name="o_stage")
            cp_eng = nc.scalar if tt % 2 else nc.vector
            if cp_eng is nc.scalar:
                cp_eng.copy(out=o_stage[:, :sz], in_=po[:, ds(lo, sz)])
            else:
                cp_eng.tensor_copy(out=o_stage[:, :sz], in_=po[:, ds(lo, sz)])
            dma_eng = nc.gpsimd if (2 * tt + (lo != 0)) % 2 else nc.sync
            dma_eng.dma_start(out[ts(tt, P), ds(lo, sz)], o_stage[:, :sz])

    if dbg:
        tiles = {"QT": QT, "KT": KT, "V": V, "OT": OT}
        for name, dap in dbg.items():
            nc.sync.dma_start(dap, tiles[name])

    for pool in (psum, singles, stream, big):
        pool.release()


_CACHE = {}


def _get_nc():
    if "nc" not in _CACHE:
        _CACHE["nc"] = build_nc()
    return _CACHE["nc"]


def _prep_inputs(x, w_qkv, b_qkv, w_out, b_out):
    import ml_dtypes

    bf16 = ml_dtypes.bfloat16
    x = np.asarray(x, np.float32)
    w_qkv = np.asarray(w_qkv, np.float32)
    b_qkv = np.asarray(b_qkv, np.float32)
    w_out = np.asarray(w_out, np.float32)
    b_out = np.asarray(b_out, np.float32)

    wq_n = w_qkv[:, 0:768]
    wk_n = w_qkv[:, 768:1536]
    wv_raw = w_qkv[:, 1536:2304]
    # [p, o] layout: contiguous per-partition rows on the wire
    bq = np.ascontiguousarray(b_qkv[0:768].reshape(DC, P).T)
    bk = np.ascontiguousarray(b_qkv[768:1536].reshape(DC, P).T)
    bv_raw = b_qkv[1536:2304]

    # [ct, p, o, c] layout so the per-ct stationary DMA is contiguous
    def w_re(w):
        return np.ascontiguousarray(
            w.reshape(DC, P, DC, P).transpose(2, 1, 0, 3).astype(bf16)
        )

    wq_r = w_re(wq_n)
    wk_r = w_re(wk_n)

    wv = np.ascontiguousarray(wv_raw.astype(bf16))
    # V bias folds into the output bias: softmax rows sum to 1.
    bo_eff = np.ascontiguousarray(
        (b_out + bv_raw @ w_out).astype(bf16).reshape(1, D)
    )
    wo = np.ascontiguousarray(w_out.astype(bf16))

    in_maps = []
    for b in range(2):
        xb = x[b]
        for g in range(4):
            xrot = np.roll(xb, -g * T, axis=0)
            xTb = np.ascontiguousarray(xrot.T.astype(bf16))
            in_maps.append(
                dict(
                    xT=xTb, wq=wq_r, wk=wk_r, wv=wv, bq=bq, bk=bk,
                    wo=wo, bo=bo_eff,
                )
            )
    return in_maps


def run_on_hw(x, w_qkv, b_qkv, w_out, b_out, **kwargs):
    in_maps = _prep_inputs(x, w_qkv, b_qkv, w_out, b_out)
    res = run_bass_kernel_spmd(_get_nc(), in_maps, core_ids=list(range(8)), **kwargs)
    full = np.empty((2, 2048, D), np.float32)
    for b in range(2):
        for g in range(4):
            full[b, g * T : (g + 1) * T] = np.asarray(
                res.results[b * 4 + g]["out"], np.float32
            )
    return full, res


def kernel(x, w_qkv, b_qkv, w_out, b_out):
    full, _ = run_on_hw(x, w_qkv, b_qkv, w_out, b_out)
    return full



# revision 27
# speedup vs baseline: 1.4235x; 1.0359x over previous
"""Distributed multi-head attention forward for 8 TRN2 NeuronCores.

Problem: B=2, N=2048, D=768, 12 heads x 64 head-dim, f32.
  qkv = x @ w_qkv + b_qkv ; per-head softmax(q k^T / 8) v ; out proj.

Sharding: core = 4*b + g (b = batch element, g = query-chunk of 512 rows).
No collectives: every core receives the FULL x^T of its batch (bf16,
host-transposed, token-rotated so its own 512 query rows sit first) and
replicates the K^T / V projections for all 2048 keys locally — on this part
the 55us+ fixed cost of a 4-core ring AllGather loses to ~60us of extra
bf16 matmuls that pipeline perfectly.

Schedule (single PE stream, everything else slotted around it):
  Q proj -> K proj ct 0 -> attention j=0..4 each interleaving the next K
  column block as PE filler (j=0 also interleaves all 16 V-projection
  steps chunk-by-chunk) -> attention j=5 -> output projection.  S runs two
  chunks ahead (PSUM: S tiles 3-deep = 6 banks + one PV accumulator pair =
  2 banks); each head pair's finalize (den -> ones-broadcast matmul ->
  reciprocal_approx_fast -> multiply, all off the Scalar engine so it does
  exps only) is deferred into chunk 0 of the next pair's loop.

Layouts: all activations transposed ([cols, tokens]) except V (natural),
everything bf16 on the wire and in SBUF; psum accumulation f32.  V carries
a per-head ones column so P@V also yields the softmax denominator; the V
bias is folded into the output bias on the host (sum(P)=1).
"""

import numpy as np

import concourse.bass as bass
import concourse.tile as tile
from concourse import bacc, mybir
from concourse.bass import ts, ds
from concourse.bass_utils import run_bass_kernel_spmd

FP = mybir.dt.float32
FR = mybir.dt.float32r
BF = mybir.dt.bfloat16

P = 128
T = 512            # query rows per core
D = 768            # model dim
H = 12             # heads
DH = 64            # head dim
VA = H * (DH + 1)  # 780: v columns + per-head ones column (softmax den)
VH = 6 * (DH + 1)  # one half (6 heads) of the V columns
KEYS = 2048
DC = D // P        # 6 chunks of the contraction dim
NKC = KEYS // P    # 16 key chunks of 128
NKT = KEYS // T    # 4 key chunks of 512
SCALE = DH ** -0.5


def build_nc():
    nc = bacc.Bacc(
        "TRN2",
        target_bir_lowering=False,
        debug=False,
        enable_asserts=False,
        num_devices=8,
    )
    import os
    dbg = {}
    for name, shape in (
        ("dQT", [P, DC, T]), ("dKT", [P, DC, KEYS]),
        ("dV", [P, NKC, VA]), ("dOT", [P, DC, T]),
    ):
        if name[1:] in os.environ.get("KDBG", "").split(","):
            dbg[name[1:]] = nc.dram_tensor(name, shape, BF, kind="ExternalOutput").ap()

    xT = nc.dram_tensor("xT", [D, KEYS], BF, kind="ExternalInput").ap()
    wq = nc.dram_tensor("wq", [DC, P, DC, P], BF, kind="ExternalInput").ap()
    wk = nc.dram_tensor("wk", [DC, P, DC, P], BF, kind="ExternalInput").ap()
    wv = nc.dram_tensor("wv", [D, D], BF, kind="ExternalInput").ap()
    bq = nc.dram_tensor("bq", [P, DC], FP, kind="ExternalInput").ap()
    bk = nc.dram_tensor("bk", [P, DC], FP, kind="ExternalInput").ap()
    wo = nc.dram_tensor("wo", [D, D], BF, kind="ExternalInput").ap()
    bo = nc.dram_tensor("bo", [1, D], BF, kind="ExternalInput").ap()
    out = nc.dram_tensor("out", [T, D], FP, kind="ExternalOutput").ap()

    with tile.TileContext(nc) as tc:
        _build_body(tc, xT, wq, wk, wv, bq, bk, wo, bo, out, dbg)
    nc.compile()
    return nc


def _build_body(tc, xT_d, wq, wk, wv, bq, bk, wo, bo, out, dbg=None):
    nc = tc.nc
    Add = mybir.AluOpType.add
    Mult = mybir.AluOpType.mult
    Exp = mybir.ActivationFunctionType.Exp

    big = tc.alloc_tile_pool(name="big", bufs=1)
    stream = tc.alloc_tile_pool(name="stream", bufs=2)
    singles = tc.alloc_tile_pool(name="singles", bufs=1)
    psum = tc.alloc_tile_pool(name="psum", bufs=2, space="PSUM")

    # b2: [128, 1024] f32 = 2 psum banks; bufs=3 -> 6 banks.
    def b2(name):
        return psum.tile([P, 2 * T], FP, tag="b2", bufs=3, name=name)

    # pv: attention accumulator, 2 banks, single-buffered.
    def bpv(name):
        return psum.tile([P, 2 * T], FP, tag="pv", bufs=1, name=name)

    # ---- persistent SBUF tensors ----
    xT = big.tile([P, DC, KEYS], BF)     # x^T, all tokens (rotated)
    QT = big.tile([P, DC, T], BF)        # Q^T for own 512 rows (biased)
    KT = big.tile([P, DC, KEYS], BF)     # K^T all keys (biased)
    V = big.tile([P, NKC, VA], BF)       # V all keys (+ones cols)
    OT = big.tile([P, DC, T], BF)        # attention output, transposed
    wv_sb = big.tile([P, DC, D], BF)
    wo_sb = big.tile([P, DC, D], BF)

    # ---- constants ----
    ones_bf = singles.tile([1, DH], BF)
    nc.vector.memset(ones_bf, 1.0)
    ones_row = singles.tile([1, P], BF)   # K=1 stationary for the bias matmul
    nc.vector.memset(ones_row, 1.0)

    junk = singles.tile([P, P], BF)       # PE warm-up operand, contents unused
    nc.vector.memset(junk, 0.0)
    bq_sb = singles.tile([P, DC], FP)
    bk_sb = singles.tile([P, DC], FP)
    bo_row = singles.tile([1, D], BF)

    # ---- PE warm-up: junk matmuls with no DMA deps so the HAM un-throttles
    # and the array is at 2.4 GHz when the first real matmul's inputs land.
    warm_ps = b2("warm")
    for _ in range(44):
        nc.tensor.matmul(warm_ps[:, :P], junk, junk, start=True, stop=True)

    # ---- input DMAs: split fine-grained, spread across engine queues, in
    # consumption order (descriptor issue is ~0.6-0.8us per dma_start and
    # serializes per engine; the old single-queue scheme pushed the first
    # matmul's deps out to ~17us).
    wq_sb = big.tile([P, DC, DC, P], BF)   # [p, ct, o, c]
    wk_sb = big.tile([P, DC, DC, P], BF)
    xTr = xT_d.rearrange("(dc p) n -> p dc n", p=P)
    # wave 1: Q-proj deps (wq per-ct, x own rows per-dc, biases)
    nc.scalar.dma_start(wq_sb[:, 0], wq[0])
    nc.gpsimd.dma_start(xT[:, 0, 0:T], xTr[:, 0, 0:T])
    nc.sync.dma_start(xT[:, 1, 0:T], xTr[:, 1, 0:T])
    nc.scalar.dma_start(wq_sb[:, 1], wq[1])
    nc.gpsimd.dma_start(wq_sb[:, 2], wq[2])
    nc.sync.dma_start(xT[:, 2, 0:T], xTr[:, 2, 0:T])
    nc.scalar.dma_start(bq_sb, bq)
    nc.gpsimd.dma_start(xT[:, 3, 0:T], xTr[:, 3, 0:T])
    nc.sync.dma_start(wq_sb[:, 3], wq[3])
    nc.scalar.dma_start(bk_sb, bk)
    nc.gpsimd.dma_start(xT[:, 4, 0:T], xTr[:, 4, 0:T])
    nc.sync.dma_start(xT[:, 5, 0:T], xTr[:, 5, 0:T])
    nc.gpsimd.dma_start(wq_sb[:, 4], wq[4])
    nc.sync.dma_start(wq_sb[:, 5], wq[5])
    # wave 2: K ct0 weights + x remaining keys (kc>=1), then V/out weights
    nc.sync.dma_start(wk_sb[:, 0], wk[0])
    for dc in range(DC):
        eng = nc.gpsimd if dc % 2 else nc.sync
        eng.dma_start(wv_sb[:, dc, :], wv[ts(dc, P), :])
    # x keys 512:1024 first (pair-0's k/v fills need them early), rest after
    for dc in range(DC):
        eng = nc.gpsimd if dc % 2 else nc.sync
        eng.dma_start(xT[:, dc, T : 2 * T], xT_d[ts(dc, P), T : 2 * T])
    for dc in range(DC):
        eng = nc.gpsimd if dc % 2 else nc.sync
        eng.dma_start(xT[:, dc, 2 * T : KEYS], xT_d[ts(dc, P), 2 * T : KEYS])
    for ct in range(1, DC):
        eng = nc.gpsimd if ct % 2 else nc.sync
        eng.dma_start(wk_sb[:, ct], wk[ct])
    for dc in range(DC):
        eng = nc.gpsimd if dc % 2 else nc.sync
        eng.dma_start(wo_sb[:, dc, :], wo[ts(dc, P), :])
    nc.gpsimd.dma_start(bo_row, bo)

    # ---- phase 1: Q^T projection; ct 0-1 upfront, the rest interleaved ----
    def q_group(ct):
        pq = b2("pq")
        for dc in range(DC):
            nc.tensor.matmul(
                pq[:, :T], wq_sb[:, ct, dc, :], xT[:, dc, 0:T],
                start=(dc == 0), stop=(dc == DC - 1),
            )
        nc.scalar.add(QT[:, ct, :], pq[:, :T], bq_sb[:, ct : ct + 1])

    for ct in range(DC):
        q_group(ct)

    # ---- phase 2: K^T projection; ct 0-1 upfront, ct 2-5 interleaved into
    # the attention loop as PE filler work.
    def k_group(ct, kc):
        pk = b2("pk")
        for dc in range(DC):
            nc.tensor.matmul(
                pk[:, :T], wk_sb[:, ct, dc, :], xT[:, dc, ts(kc, T)],
                start=(dc == 0), stop=(dc == DC - 1),
            )
        nc.vector.tensor_scalar(
            out=KT[:, ct, ts(kc, T)], in0=pk[:, :T],
            scalar1=bk_sb[:, ct : ct + 1], scalar2=None, op0=Add,
        )

    k_group(0, 0)

    # ---- phase 3+4: V projection, split in half by head group: pair j only
    # consumes its own heads' V columns, so heads 0-5 are produced just-in-time
    # inside pair 0's chunk loop and heads 6-11 inside pair 1's.  This halves
    # the V overhang of the first (V-bound) pair and hides the second half in
    # pair 1, whose chunks are otherwise exp-bound.
    def v_step(tt, half):
        pv = b2("pvproj")
        lo = half * (D // 2)
        for dc in range(DC):
            nc.tensor.matmul(
                pv[:, 0 : D // 2],
                xT[:, dc, ts(tt, P)],
                wv_sb[:, dc, ds(lo, D // 2)],
                start=(dc == 0), stop=(dc == DC - 1),
            )
        dst = V[:, tt, ds(half * VH, VH)].rearrange("p (h d1) -> p h d1", d1=DH + 1)
        nc.vector.tensor_copy(
            out=dst[:, :, 0:DH],
            in_=pv[:, 0 : D // 2].rearrange("p (h d) -> p h d", d=DH),
        )
        nc.vector.memset(dst[:, :, DH], 1.0)

    def attn_j(j, v_half=None, fill_k=(), fill_mod=4, fill_min=1, fin_prev=None):
        """Attention for head pair (2j, 2j+1) over all 16 key chunks.
        Returns a finalize closure (run it one j later to pipeline).
        If v_half is set, that half of the V-projection is interleaved;
        fill_k closures are spread across the chunk loop as PE filler."""
        fill_k = list(fill_k)
        pv_acc = None  # allocated lazily at the first PV accumulation
        ps_tiles = {}

        def s_step(c):
            ps = b2(f"ps{j}_{c}")
            ps_tiles[c] = ps
            for hl, off in ((0, 0), (1, DH)):
                nc.tensor.matmul(
                    ps[:, ds(hl * T, T)],
                    KT[ds(off, DH), j, ts(c, P)],
                    QT[ds(off, DH), j, :],
                    start=True, stop=True,
                )

        # V-interleaved pairs use a 1-chunk S lookahead: with the V-projection
        # also allocating from b2, a 2-ahead emission makes v(c+2) wait on a
        # future exp via the 3-buffer rotation.  Pure-attention pairs use 2.
        ahead = 1 if v_half is not None else 2
        for c0 in range(ahead):
            if v_half is not None:
                v_step(c0, v_half)
            s_step(c0)
        for c in range(NKC):
            es = stream.tile([P, 2 * T], BF, tag="expS", bufs=5, name="es")
            nc.scalar.activation(es, ps_tiles[c][:, :], Exp, scale=SCALE)
            if c == 0 and fin_prev is not None:
                fin_prev()
            # fills precede the S lookahead: a fill's output may feed the
            # s_step emitted in the same iteration (pair 0's own K chunks)
            if fill_k and c % fill_mod == 1 and c >= fill_min:
                fill_k.pop(0)()
            if c + ahead < NKC:
                s_step(c + ahead)
                if v_half is not None:
                    v_step(c + ahead, v_half)
            if pv_acc is None:
                pv_acc = bpv(f"pv{j}")  # h0: [:65, :512], h1: [:65, 512:]
            for hl in (0, 1):
                nc.tensor.matmul(
                    pv_acc[: DH + 1, ds(hl * T, T)],
                    V[:, c, ds((2 * j + hl) * (DH + 1), DH + 1)],
                    es[:, ds(hl * T, T)],
                    start=(c == 0), stop=(c == NKC - 1),
                )

        def finalize():
            den_bf = stream.tile([1, 2 * T], BF, tag="den", bufs=2, name="den_bf")
            nc.vector.tensor_copy(out=den_bf, in_=pv_acc[DH : DH + 1, :])
            bc = b2(f"bc{j}")
            for hl in (0, 1):
                nc.tensor.matmul(
                    bc[:DH, ds(hl * T, T)], ones_bf, den_bf[:, ds(hl * T, T)],
                    start=True, stop=True,
                )
            recip = stream.tile([DH, 2 * T], FP, tag="recip", bufs=2, name="recip")
            nc.vector.reciprocal_approx_fast(out=recip, in_=bc[:DH, :])
            for hl in (0, 1):
                nc.vector.tensor_tensor(
                    out=OT[ds(hl * DH, DH), j, :],
                    in0=pv_acc[:DH, ds(hl * T, T)],
                    in1=recip[:, ds(hl * T, T)], op=Mult,
                )

        return finalize

    def po_head(tt):
        po = b2(f"po{tt}")
        for lo, sz in ((0, T), (T, D - T)):
            # bias first (K=1 ones matmul) so the tail is just dc=5 + copy
            nc.tensor.matmul(
                po[:, ds(lo, sz)], ones_row, bo_row[:, ds(lo, sz)],
                start=True, stop=False,
            )
        for dc in range(DC - 1):
            for lo, sz in ((0, T), (T, D - T)):
                nc.tensor.matmul(
                    po[:, ds(lo, sz)],
                    OT[:, dc, ts(tt, P)],
                    wo_sb[:, dc, ds(lo, sz)],
                    start=False, stop=False,
                )
        return po

    po_tiles = {}

    def po_fill(tt):
        po_tiles[tt] = po_head(tt)

    fin = None
    for j in range(DC):
        if j == 0:
            # k ct0 kc1-3 first (pair 0's own S consumes them), then ct1;
            # fills start at c=3 so the x-rest/wv DMAs have landed and a
            # stalled fill cannot block the PE queue behind it.
            fill_k = tuple(
                (lambda kc=kc: k_group(0, kc)) for kc in range(1, NKT)
            ) + tuple(
                (lambda kc=kc: k_group(1, kc)) for kc in range(NKT)
            )
            fin = attn_j(j, v_half=0, fill_k=fill_k, fill_mod=2, fill_min=3,
                         fin_prev=fin)
        elif j == 1:
            fill_k = tuple(
                (lambda kc=kc: k_group(2, kc)) for kc in range(NKT)
            )
            fin = attn_j(j, v_half=1, fill_k=fill_k, fin_prev=fin)
        elif j <= 4:
            ct = j + 1
            fill_k = tuple(
                (lambda ct=ct, kc=kc: k_group(ct, kc)) for kc in range(NKT)
            )
            fin = attn_j(j, fill_k=fill_k, fin_prev=fin)
        else:
            fin = attn_j(j, fin_prev=fin)

    # ---- phase 6: output projection.  The dc 0-4 partial chains for the
    # first two token tiles don't need OT[:, 5], so they run before (and
    # overlap) the last head pair's finalize chain.
    po_fill(0)
    po_fill(1)
    fin()
    for tt in range(T // P):
        if tt not in po_tiles:
            po_tiles[tt] = po_head(tt)
        po = po_tiles[tt]
        for lo, sz in ((0, T), (T, D - T)):
            nc.tensor.matmul(
                po[:, ds(lo, sz)],
                OT[:, DC - 1, ts(tt, P)],
                wo_sb[:, DC - 1, ds(lo, sz)],
                start=False, stop=True,
            )
        for lo, sz in ((0, T), (T, D - T)):
            o_stage = stream.tile([P, T], FP, tag="ost", bufs=4, name="o_stage")
            cp_eng = nc.scalar if tt % 2 else nc.vector
            if cp_eng is nc.scalar:
                cp_eng.copy(out=o_stage[:, :sz], in_=po[:, ds(lo, sz)])
            else:
                cp_eng.tensor_copy(out=o_stage[:, :sz], in_=po[:, ds(lo, sz)])
            dma_eng = nc.gpsimd if (2 * tt + (lo != 0)) % 2 else nc.sync
            dma_eng.dma_start(out[ts(tt, P), ds(lo, sz)], o_stage[:, :sz])

    if dbg:
        tiles = {"QT": QT, "KT": KT, "V": V, "OT": OT}
        for name, dap in dbg.items():
            nc.sync.dma_start(dap, tiles[name])

    for pool in (psum, singles, stream, big):
        pool.release()


_CACHE = {}


def _get_nc():
    if "nc" not in _CACHE:
        _CACHE["nc"] = build_nc()
    return _CACHE["nc"]


def _prep_inputs(x, w_qkv, b_qkv, w_out, b_out):
    import ml_dtypes

    bf16 = ml_dtypes.bfloat16
    x = np.asarray(x, np.float32)
    w_qkv = np.asarray(w_qkv, np.float32)
    b_qkv = np.asarray(b_qkv, np.float32)
    w_out = np.asarray(w_out, np.float32)
    b_out = np.asarray(b_out, np.float32)

    wq_n = w_qkv[:, 0:768]
    wk_n = w_qkv[:, 768:1536]
    wv_raw = w_qkv[:, 1536:2304]
    # [p, o] layout: contiguous per-partition rows on the wire
    bq = np.ascontiguousarray(b_qkv[0:768].reshape(DC, P).T)
    bk = np.ascontiguousarray(b_qkv[768:1536].reshape(DC, P).T)
    bv_raw = b_qkv[1536:2304]

    # [ct, p, o, c] layout so the per-ct stationary DMA is contiguous
    def w_re(w):
        return np.ascontiguousarray(
            w.reshape(DC, P, DC, P).transpose(2, 1, 0, 3).astype(bf16)
        )

    wq_r = w_re(wq_n)
    wk_r = w_re(wk_n)

    wv = np.ascontiguousarray(wv_raw.astype(bf16))
    # V bias folds into the output bias: softmax rows sum to 1.
    bo_eff = np.ascontiguousarray(
        (b_out + bv_raw @ w_out).astype(bf16).reshape(1, D)
    )
    wo = np.ascontiguousarray(w_out.astype(bf16))

    in_maps = []
    for b in range(2):
        xb = x[b]
        for g in range(4):
            xrot = np.roll(xb, -g * T, axis=0)
            xTb = np.ascontiguousarray(xrot.T.astype(bf16))
            in_maps.append(
                dict(
                    xT=xTb, wq=wq_r, wk=wk_r, wv=wv, bq=bq, bk=bk,
                    wo=wo, bo=bo_eff,
                )
            )
    return in_maps


def run_on_hw(x, w_qkv, b_qkv, w_out, b_out, **kwargs):
    in_maps = _prep_inputs(x, w_qkv, b_qkv, w_out, b_out)
    res = run_bass_kernel_spmd(_get_nc(), in_maps, core_ids=list(range(8)), **kwargs)
    full = np.empty((2, 2048, D), np.float32)
    for b in range(2):
        for g in range(4):
            full[b, g * T : (g + 1) * T] = np.asarray(
                res.results[b * 4 + g]["out"], np.float32
            )
    return full, res


def kernel(x, w_qkv, b_qkv, w_out, b_out):
    full, _ = run_on_hw(x, w_qkv, b_qkv, w_out, b_out)
    return full

